# revision 27
# baseline (speedup 1.0000x reference)
"""Trainium2 Bass kernel for the DLI (dialogue-turn ordering) loss.

Math (exact reduction of the reference):
  With 2 classes, NLL(label y) = softplus(l_{1-y} - l_y).
  u[b,j] = enc[b,j] @ (W[:D,1]-W[:D,0]),
  v[b,k] = enc[b,k] @ (W[D:,1]-W[D:,0]),
  c      = b[1]-b[0],  d[b,j,k] = u[b,j] + v[b,k] + c
  label = 1 iff k == j-1; valid pairs: k < j < len_b;  softplus(-d) = softplus(d) - d
  =>  sum_nll = sum_{valid} softplus(d) - sum_{b, 1<=j<len_b} d[b,j,j-1]
  loss = sum_nll / max(n_valid, 1)

Sharding: data-parallel over batch (64 -> 8 cores x 8). Each core emits one
f32 partial sum; the host adds the 8 partials and divides by the exact
n_valid from the mask. The tiny derived tensors (wu/wv rows in bf16, the
additive row masks rmM = [j<len]?c:NEG+c and diagonal validity masks rm1/rm2)
are precomputed on the host and shipped as small extra inputs — the 64MB
encoder tensor is the memory-bound payload and stays on-device.

Engine split per core (target_regime=memory; HBM floor ~23us for 8MB):
  DMA:    enc arrives bf16 via SWDGE casting DMAs (HBM reads stay f32/8MB);
          w rows broadcast down partitions by a stride-0-source DMA
  DVE:    6 u-dots as fused STT(+accum); bf16 2x-mode multiplies for the
          other 10 dots; small phase-B masking ops
  ACT:    10 dot reductions (Copy+accum_out), per-pair Exp (bias folds
          u+rowmask+c, reads PSUM) and Ln(x+1) with fused row-sum
  PE:     v-column transposes + broadcast matmuls building d[j,k] in PSUM
  GpSimd: SWDGE DMA triggers + one-time memset/affine consts only (no ucode
          ops: they force Q7 dge-drains against in-flight SWDGE DMAs, and big
          GpSimd tensor ops crawl while DVE streams -- shared SBUF ports)
All ACT functions (Copy/Exp/Ln) are forced into the single
natural_log_exp_and_others table so the act-table is loaded exactly once.
"""

import glob
import json
import os
import shutil
import sys
import tempfile

if "/opt/trn_rl_repo" not in sys.path:
    sys.path.insert(0, "/opt/trn_rl_repo")


def _force_combined_act_table():
    """Point walrus at an act_info.json holding only natural_log_exp_and_others
    (contains exp+ln+copy), so every ACTIVATE shares one table."""
    if os.environ.get("BASS_ACT_ROOT_JSON_PATH"):
        return
    from neuronxcc.driver.Job import Job  # type: ignore

    pwp = None
    for cand in glob.glob(os.path.join(Job.getPackageDir(), "pwp", "pwp_bin_*")):
        if os.path.exists(os.path.join(cand, "act_info.json")):
            pwp = cand
            break
    if pwp is None:
        return
    info = json.load(open(os.path.join(pwp, "act_info.json")))
    keep = [t for t in info.get("act_func_sets", [])
            if t.get("name") == "natural_log_exp_and_others"]
    if not keep:
        return
    out_dir = os.path.join(tempfile.gettempdir(), "dli_act_combined")
    os.makedirs(out_dir, exist_ok=True)
    for t in keep:
        for k in info.get("pwp_file_keys", []):
            f = t.get(k)
            src = os.path.join(pwp, f) if f else None
            if src and os.path.exists(src):
                dst = os.path.join(out_dir, f)
                if not os.path.exists(dst):
                    shutil.copy(src, dst)
    info = dict(info)
    info["act_func_sets"] = keep
    with open(os.path.join(out_dir, "act_info.json"), "w") as f:
        json.dump(info, f)
    os.environ["BASS_ACT_ROOT_JSON_PATH"] = os.path.join(out_dir, "act_info.json")


_force_combined_act_table()

from contextlib import ExitStack

import ml_dtypes
import numpy as np

import concourse.bacc as bacc
import concourse.bass as bass
import concourse.hw_specs as hw_specs
import concourse.mybir as mybir
import concourse.tile as tile
from concourse.masks import make_identity

# Make bass's act-table placement agree with the trimmed act_info.json walrus
# sees: only the combined exp+ln+copy table exists, so every ACTIVATE maps to
# act_func_set_id 0 and the table is loaded exactly once.
_orig_get_act_tables = hw_specs.get_activation_tables


def _combined_act_tables(module_arch):
    tabs = _orig_get_act_tables(module_arch)
    kept = {k: v for k, v in tabs.items() if k == "natural_log_exp_and_others"}
    return kept if kept and os.environ.get("BASS_ACT_ROOT_JSON_PATH") else tabs


hw_specs.get_activation_tables = _combined_act_tables
bacc.get_activation_tables = _combined_act_tables

# Cheaper kernel teardown: drain + one all-engine barrier + sem clear. The
# stock epilogue adds a second all-engine barrier after the clear; engines
# that pass the first barrier only run their terminal branch, and the next
# execution starts only after every engine (incl. the clearing one) halts,
# so the second barrier only adds ~4us of EVSEM latency.
from concourse.vector_clock import ScopedClock as _ScopedClock


def _cheap_drain_and_barrier(self, tick_clock, wait_clock):
    drain_inst = self.nc.sync.drain()
    wait_clock.add_sem_waits(
        drain_inst.ins, _ScopedClock({None: tick_clock.global_clock})
    )
    self.nc.all_engine_barrier()
    popped = self.nc._tile_sem_poison_stack.pop()
    assert popped is self._sem_poison
    self.nc.clear_and_free_semaphores(list(self.sems.allocated().values()))


tile.TileContext._drain_and_barrier = _cheap_drain_and_barrier

F32 = mybir.dt.float32
BF16 = mybir.dt.bfloat16
ALU = mybir.AluOpType
ACTF = mybir.ActivationFunctionType
AXX = mybir.AxisListType.X

BSZ, L, D = 64, 128, 2048
N_CORES = 8
NB = BSZ // N_CORES  # batches per core
NEG = -30000.0  # additive row-mask value; exp(NEG) == 0 in f32
STT_SET = {0, 1, 2}  # u-dots as fused DVE STT (rest: DVE mul + ACT accum)
V_STT_SET = {6, 7}  # v-dots as DVE STT so the tail isn't gated on ACT reduces
CHUNKS = [1, 1, 2, 2, 1, 1]  # batches per DMA chunk (small at both ends)
LN_GROUPS = [[0, 1, 2, 3], [4, 5, 6], [7]]  # batched Ln reductions (last alone: short tail)


def build_program():
    nc = bacc.Bacc("TRN2", target_bir_lowering=False, debug=False, num_devices=1)

    enc = nc.dram_tensor("enc", [NB, L, D], F32, kind="ExternalInput").ap()
    wuv = nc.dram_tensor("wuv", [2, D], BF16, kind="ExternalInput").ap()
    aux = nc.dram_tensor("aux", [L, 3 * NB], F32, kind="ExternalInput").ap()
    out = nc.dram_tensor("out", [1, 1], F32, kind="ExternalOutput").ap()

    with tile.TileContext(nc) as tc, ExitStack() as ctx:
        consts = ctx.enter_context(tc.tile_pool(name="consts", bufs=1))
        accs = ctx.enter_context(tc.tile_pool(name="accs", bufs=1))
        enc_pool = ctx.enter_context(tc.tile_pool(name="enc", bufs=3))
        junk_pool = ctx.enter_context(tc.tile_pool(name="junk", bufs=3))
        prod_pool = ctx.enter_context(tc.tile_pool(name="prod", bufs=4))
        d2_pool = ctx.enter_context(tc.tile_pool(name="d2", bufs=3))
        rows_pool = ctx.enter_context(tc.tile_pool(name="rows", bufs=3))
        psum_d_pool = ctx.enter_context(tc.tile_pool(name="psd", bufs=3, space="PSUM"))
        psum_v_pool = ctx.enter_context(tc.tile_pool(name="psv", bufs=2, space="PSUM"))
        psum_misc = ctx.enter_context(tc.tile_pool(name="psm", bufs=1, space="PSUM"))

        # ---- w rows: tiny HBM fetch, broadcast via PE matmul + ACT copies ----
        # (PE/ACT are idle during the ramp; avoids re-reading 1MB from HBM,
        # which shares the bus pacing the 8MB enc stream)
        wv_sb = consts.tile([1, D], BF16)
        nc.sync.dma_start(wv_sb[:], wuv[1:2, :])
        wu_sb = consts.tile([1, D], BF16)
        nc.sync.dma_start(wu_sb[:], wuv[0:1, :])
        ones_row_bf = consts.tile([1, L], BF16)
        nc.gpsimd.memset(ones_row_bf[:], 1.0)
        wv_b = consts.tile([L, D], BF16)
        wu_b = consts.tile([L, D], BF16)
        for w_b, w_sb, nm in ((wv_b, wv_sb, "v"), (wu_b, wu_sb, "u")):
            for cq in range(D // 512):
                psum_w = psum_v_pool.tile([L, 512], F32, tag="psw",
                                          name=f"psw{nm}{cq}")
                nc.tensor.matmul(psum_w[:], lhsT=ones_row_bf[:],
                                 rhs=w_sb[0:1, cq * 512 : (cq + 1) * 512])
                nc.scalar.copy(w_b[:, cq * 512 : (cq + 1) * 512], psum_w[:])
        aux_sb = consts.tile([L, 3 * NB], F32)
        nc.sync.dma_start(aux_sb[:], aux[:])
        rmM_all = aux_sb[:, 0:NB]          # (j<len_b ? 0 : NEG) + c
        rm1_all = aux_sb[:, NB : 2 * NB]   # [1<=j<len_b]
        rm2_all = aux_sb[:, 2 * NB : 3 * NB]  # [j<len_b-1]

        # ---- enc loads: SWDGE casting DMAs, CHUNKS batches per chunk ----
        enc_tiles = {}
        chunk_of = {}
        b0 = 0
        for ci, tb in enumerate(CHUNKS):
            chunk = enc_pool.tile([L, tb * D], BF16, tag=f"enc{ci % 3}", name=f"encc{ci}")
            if tb == 1:
                nc.gpsimd.dma_start(chunk[:], enc[b0])
            else:
                nc.gpsimd.dma_start(
                    chunk[:].rearrange("l (b d) -> l b d", b=tb),
                    enc[b0 : b0 + tb].rearrange("b l d -> l b d"),
                )
            for i in range(tb):
                enc_tiles[b0 + i] = chunk[:, i * D : (i + 1) * D]
                chunk_of[b0 + i] = ci
            b0 += tb

        # ---- constants ----
        ones_row = consts.tile([1, L], F32)
        nc.gpsimd.memset(ones_row[:], 1.0)
        ones_col = consts.tile([L, 1], F32)
        nc.gpsimd.memset(ones_col[:], 1.0)
        ident = consts.tile([L, L], F32)
        make_identity(nc, ident[:])
        # multiplicative lower-triangular mask: tri01[j,k] = 1 if k<j else 0
        tri01 = consts.tile([L, L], F32)
        nc.gpsimd.memset(tri01[:], 1.0)
        nc.gpsimd.affine_select(
            out=tri01[:], in_=tri01[:], compare_op=ALU.is_gt, fill=0.0,
            base=0, pattern=[[-1, L]], channel_multiplier=1,
        )

        # ---- main pipeline ----
        UV = accs.tile([L, 2 * NB], F32)  # cols 0..NB-1 = u_b; NB..2NB-1 = v_b
        n_groups = len(LN_GROUPS)
        grp_of = {b: (g, q) for g, grp in enumerate(LN_GROUPS) for q, b in enumerate(grp)}
        RS = accs.tile([L, n_groups], F32)
        exg_pool = ctx.enter_context(tc.tile_pool(name="exg", bufs=1))
        exg_tiles = [exg_pool.tile([L, len(grp) * L], F32, tag=f"exg{i}", name=f"exg{i}")
                     for i, grp in enumerate(LN_GROUPS)]

        def dot_stt(enc_ap, w_tile, acc_col):
            junk = junk_pool.tile([L, D], BF16)
            nc.vector.scalar_tensor_tensor(
                out=junk[:], in0=enc_ap, scalar=1.0, op0=ALU.mult,
                in1=w_tile[:], op1=ALU.mult, accum_out=acc_col,
            )

        def dot_act(enc_ap, w_tile, acc_col):
            prod = prod_pool.tile([L, D], BF16)
            nc.vector.tensor_mul(prod[:], enc_ap, w_tile[:])
            junk = junk_pool.tile([L, D], BF16, tag="junk_act")
            nc.scalar.activation(junk[:], prod[:], ACTF.Copy, accum_out=acc_col)

        def phase_b(b):
            v_col = UV[:, NB + b : NB + b + 1]
            psum_v = psum_v_pool.tile([1, L], F32)
            nc.tensor.matmul(psum_v[:], lhsT=v_col, rhs=ident[:], is_transpose=True)
            v_row = rows_pool.tile([1, L], F32, tag="vrow")
            nc.vector.tensor_copy(v_row[:], psum_v[:])
            psum_d = psum_d_pool.tile([L, L], F32)
            nc.tensor.matmul(psum_d[:], lhsT=ones_row[:], rhs=v_row[:])
            su = rows_pool.tile([L, 1], F32, tag="su")
            nc.vector.tensor_add(su[:], UV[:, b : b + 1], rmM_all[:, b : b + 1])
            ex = d2_pool.tile([L, L], F32, tag="ex")
            nc.scalar.activation(ex[:], psum_d[:], ACTF.Exp, bias=su[:, 0:1])
            g, q = grp_of[b]
            nc.vector.tensor_mul(exg_tiles[g][:, q * L : (q + 1) * L], ex[:], tri01[:])

        def ln_group(g):
            sp = d2_pool.tile([L, len(LN_GROUPS[g]) * L], F32, tag="sp")
            nc.scalar.activation(sp[:], exg_tiles[g][:], ACTF.Ln, bias=1.0,
                                 accum_out=RS[:, g : g + 1])

        done_groups = set()
        b0 = 0
        for tb in CHUNKS:
            batches = range(b0, b0 + tb)
            for b in batches:
                # v-dots first: they gate phase B. Tail batches use DVE STT
                # so phase B isn't queued behind ACT reductions.
                if b in V_STT_SET:
                    dot_stt(enc_tiles[b], wv_b, UV[:, NB + b : NB + b + 1])
                else:
                    dot_act(enc_tiles[b], wv_b, UV[:, NB + b : NB + b + 1])
            for b in batches:
                if b in STT_SET:
                    dot_stt(enc_tiles[b], wu_b, UV[:, b : b + 1])
                else:
                    dot_act(enc_tiles[b], wu_b, UV[:, b : b + 1])
            for b in batches:
                phase_b(b)
            b0 += tb
            for g, grp in enumerate(LN_GROUPS):
                if g not in done_groups and grp[-1] < b0:
                    ln_group(g)
                    done_groups.add(g)

        # ---- diagonal (label-1) terms, all batches at once ----
        # diag sum = sum_j u[j]*rm1[j] + sum_k (v[k]+c)*rm2[k]; the c*rm2 part
        # equals c*(len-1) and is folded in on the host via rmM's c... no:
        # rm2 carries plain 0/1; vc adds nothing here because c is folded into
        # rmM (bias path). The diagonal needs v+c explicitly, so the host puts
        # c into rm2's companion: we compute sum v*rm2 and the host adds
        # c*(len_b-1) terms into its final combine.
        dUV = accs.tile([L, 2 * NB], F32)
        nc.vector.tensor_mul(dUV[:], UV[:, 0 : 2 * NB], aux_sb[:, NB : 3 * NB])

        # ---- final reduction ----
        accA = accs.tile([L, 1], F32)
        nc.vector.reduce_sum(accA[:], RS[:], axis=AXX)
        dr = accs.tile([L, 1], F32)
        nc.vector.reduce_sum(dr[:], dUV[:], axis=AXX)
        nc.vector.tensor_sub(accA[:], accA[:], dr[:])
        psum_s = psum_misc.tile([1, 1], F32, tag="psm")
        nc.tensor.matmul(psum_s[:], lhsT=accA[:], rhs=ones_col[:])
        out_t = accs.tile([1, 1], F32)
        nc.vector.tensor_copy(out_t[:], psum_s[:])
        nc.sync.dma_start(out[:], out_t[:])

    nc.compile()
    return nc


_NC = None


def _get_nc():
    global _NC
    if _NC is None:
        _NC = build_program()
    return _NC


def _prep(encoder_output, mask, W, b):
    """Host-side prep: shard + derived small tensors."""
    W = np.asarray(W, dtype=np.float32)
    b = np.asarray(b, dtype=np.float32).reshape(2)
    mask = np.asarray(mask)
    c = float(b[1] - b[0])
    wuv = np.stack([W[:D, 1] - W[:D, 0], W[D:, 1] - W[D:, 0]]).astype(ml_dtypes.bfloat16)
    lens = mask.astype(np.int64).sum(axis=1)  # [BSZ]
    j = np.arange(L)
    maps = []
    diag_c = 0.0  # host part of the diagonal c-terms: sum_b c*(len_b-1)
    for cid in range(N_CORES):
        sl = slice(cid * NB, (cid + 1) * NB)
        lc = lens[sl]  # [NB]
        rmM = np.where(j[:, None] < lc[None, :], 0.0, NEG).astype(np.float32) + c
        rm1 = ((j[:, None] >= 1) & (j[:, None] < lc[None, :])).astype(np.float32)
        rm2 = (j[:, None] < (lc[None, :] - 1)).astype(np.float32)
        aux = np.concatenate([rmM, rm1, rm2], axis=1)  # [L, 3*NB]
        maps.append(
            {
                "enc": np.ascontiguousarray(encoder_output[sl], dtype=np.float32),
                "wuv": wuv,
                "aux": np.ascontiguousarray(aux),
            }
        )
    diag_c = float(c * (lens - 1).sum())
    n_valid = int((lens * (lens - 1) // 2).sum())
    return maps, diag_c, n_valid


def kernel(encoder_output, mask, W, b, _run_kwargs=None):
    from concourse.bass_utils import run_bass_kernel_spmd

    nc = _get_nc()
    maps, diag_c, n_valid = _prep(np.asarray(encoder_output), mask, W, b)
    res = run_bass_kernel_spmd(nc, maps, core_ids=list(range(N_CORES)),
                               **(_run_kwargs or {}))
    total = float(sum(np.float64(r["out"][0, 0]) for r in res.results))
    total -= diag_c
    loss = total / max(n_valid, 1)
    out = np.array(loss, dtype=np.float32)
    if _run_kwargs is not None:
        return out, res
    return out


# revision 29
# speedup vs baseline: 1.0740x; 1.0740x over previous
"""Trainium2 Bass kernel for the DLI (dialogue-turn ordering) loss.

Math (exact reduction of the reference):
  With 2 classes, NLL(label y) = softplus(l_{1-y} - l_y).
  u[b,j] = enc[b,j] @ (W[:D,1]-W[:D,0]),
  v[b,k] = enc[b,k] @ (W[D:,1]-W[D:,0]),
  c      = b[1]-b[0],  d[b,j,k] = u[b,j] + v[b,k] + c
  label = 1 iff k == j-1; valid pairs: k < j < len_b;  softplus(-d) = softplus(d) - d
  =>  sum_nll = sum_{valid} softplus(d) - sum_{b, 1<=j<len_b} d[b,j,j-1]
  loss = sum_nll / max(n_valid, 1)

Sharding: data-parallel over batch (64 -> 8 cores x 8). Each core emits one
f32 partial sum; the host adds the 8 partials and divides by the exact
n_valid from the mask. The tiny derived tensors (wu/wv rows in bf16, the
additive row masks rmM = [j<len]?c:NEG+c and diagonal validity masks rm1/rm2)
are precomputed on the host and shipped as small extra inputs — the 64MB
encoder tensor is the memory-bound payload and stays on-device.

Engine split per core (target_regime=memory; HBM floor ~23us for 8MB):
  DMA:    enc arrives bf16 via SWDGE casting DMAs (HBM reads stay f32/8MB);
          w rows broadcast down partitions by a stride-0-source DMA
  DVE:    6 u-dots as fused STT(+accum); bf16 2x-mode multiplies for the
          other 10 dots; small phase-B masking ops
  ACT:    10 dot reductions (Copy+accum_out), per-pair Exp (bias folds
          u+rowmask+c, reads PSUM) and Ln(x+1) with fused row-sum
  PE:     v-column transposes + broadcast matmuls building d[j,k] in PSUM
  GpSimd: SWDGE DMA triggers + one-time memset/affine consts only (no ucode
          ops: they force Q7 dge-drains against in-flight SWDGE DMAs, and big
          GpSimd tensor ops crawl while DVE streams -- shared SBUF ports)
All ACT functions (Copy/Exp/Ln) are forced into the single
natural_log_exp_and_others table so the act-table is loaded exactly once.
"""

import glob
import json
import os
import shutil
import sys
import tempfile

if "/opt/trn_rl_repo" not in sys.path:
    sys.path.insert(0, "/opt/trn_rl_repo")


def _force_combined_act_table():
    """Point walrus at an act_info.json holding only natural_log_exp_and_others
    (contains exp+ln+copy), so every ACTIVATE shares one table."""
    if os.environ.get("BASS_ACT_ROOT_JSON_PATH"):
        return
    from neuronxcc.driver.Job import Job  # type: ignore

    pwp = None
    for cand in glob.glob(os.path.join(Job.getPackageDir(), "pwp", "pwp_bin_*")):
        if os.path.exists(os.path.join(cand, "act_info.json")):
            pwp = cand
            break
    if pwp is None:
        return
    info = json.load(open(os.path.join(pwp, "act_info.json")))
    keep = [t for t in info.get("act_func_sets", [])
            if t.get("name") == "natural_log_exp_and_others"]
    if not keep:
        return
    out_dir = os.path.join(tempfile.gettempdir(), "dli_act_combined")
    os.makedirs(out_dir, exist_ok=True)
    for t in keep:
        for k in info.get("pwp_file_keys", []):
            f = t.get(k)
            src = os.path.join(pwp, f) if f else None
            if src and os.path.exists(src):
                dst = os.path.join(out_dir, f)
                if not os.path.exists(dst):
                    shutil.copy(src, dst)
    info = dict(info)
    info["act_func_sets"] = keep
    with open(os.path.join(out_dir, "act_info.json"), "w") as f:
        json.dump(info, f)
    os.environ["BASS_ACT_ROOT_JSON_PATH"] = os.path.join(out_dir, "act_info.json")


_force_combined_act_table()

from contextlib import ExitStack

import ml_dtypes
import numpy as np

import concourse.bacc as bacc
import concourse.bass as bass
import concourse.hw_specs as hw_specs
import concourse.mybir as mybir
import concourse.tile as tile
from concourse.masks import make_identity

# Make bass's act-table placement agree with the trimmed act_info.json walrus
# sees: only the combined exp+ln+copy table exists, so every ACTIVATE maps to
# act_func_set_id 0 and the table is loaded exactly once.
_orig_get_act_tables = hw_specs.get_activation_tables


def _combined_act_tables(module_arch):
    tabs = _orig_get_act_tables(module_arch)
    kept = {k: v for k, v in tabs.items() if k == "natural_log_exp_and_others"}
    return kept if kept and os.environ.get("BASS_ACT_ROOT_JSON_PATH") else tabs


hw_specs.get_activation_tables = _combined_act_tables
bacc.get_activation_tables = _combined_act_tables

# Cheaper kernel teardown: drain + one all-engine barrier + sem clear. The
# stock epilogue adds a second all-engine barrier after the clear; engines
# that pass the first barrier only run their terminal branch, and the next
# execution starts only after every engine (incl. the clearing one) halts,
# so the second barrier only adds ~4us of EVSEM latency.
from concourse.vector_clock import ScopedClock as _ScopedClock


def _cheap_drain_and_barrier(self, tick_clock, wait_clock):
    drain_inst = self.nc.sync.drain()
    wait_clock.add_sem_waits(
        drain_inst.ins, _ScopedClock({None: tick_clock.global_clock})
    )
    self.nc.all_engine_barrier()
    popped = self.nc._tile_sem_poison_stack.pop()
    assert popped is self._sem_poison
    self.nc.clear_and_free_semaphores(list(self.sems.allocated().values()))


tile.TileContext._drain_and_barrier = _cheap_drain_and_barrier

F32 = mybir.dt.float32
BF16 = mybir.dt.bfloat16
ALU = mybir.AluOpType
ACTF = mybir.ActivationFunctionType
AXX = mybir.AxisListType.X

BSZ, L, D = 64, 128, 2048
N_CORES = 8
NB = BSZ // N_CORES  # batches per core
NEG = -30000.0  # additive row-mask value; exp(NEG) == 0 in f32
STT_SET = {0, 1, 2}  # u-dots as fused DVE STT (rest: DVE mul + ACT accum)
V_STT_SET = {6, 7}  # v-dots as DVE STT so the tail isn't gated on ACT reduces
CHUNKS = [1, 1, 2, 2, 1, 1]  # batches per DMA chunk (small at both ends)
LN_GROUPS = [[0, 1, 2, 3], [4, 5, 6], [7]]  # batched Ln reductions (last alone: short tail)


def build_program():
    nc = bacc.Bacc("TRN2", target_bir_lowering=False, debug=False, num_devices=1)

    enc = nc.dram_tensor("enc", [NB, L, D], F32, kind="ExternalInput").ap()
    wuv = nc.dram_tensor("wuv", [2, D], BF16, kind="ExternalInput").ap()
    aux = nc.dram_tensor("aux", [L, 3 * NB], F32, kind="ExternalInput").ap()
    out = nc.dram_tensor("out", [1, 1], F32, kind="ExternalOutput").ap()

    with tile.TileContext(nc) as tc, ExitStack() as ctx:
        consts = ctx.enter_context(tc.tile_pool(name="consts", bufs=1))
        accs = ctx.enter_context(tc.tile_pool(name="accs", bufs=1))
        enc_pool = ctx.enter_context(tc.tile_pool(name="enc", bufs=3))
        junk_pool = ctx.enter_context(tc.tile_pool(name="junk", bufs=3))
        prod_pool = ctx.enter_context(tc.tile_pool(name="prod", bufs=4))
        d2_pool = ctx.enter_context(tc.tile_pool(name="d2", bufs=3))
        rows_pool = ctx.enter_context(tc.tile_pool(name="rows", bufs=3))
        psum_d_pool = ctx.enter_context(tc.tile_pool(name="psd", bufs=3, space="PSUM"))
        psum_v_pool = ctx.enter_context(tc.tile_pool(name="psv", bufs=2, space="PSUM"))
        psum_misc = ctx.enter_context(tc.tile_pool(name="psm", bufs=1, space="PSUM"))

        # ---- w rows broadcast down all partitions via stride-0-source DMAs ----
        wv_b = consts.tile([L, D], BF16)
        nc.sync.dma_start(wv_b[:], wuv[1:2, :].broadcast_to([L, D]))
        wu_b = consts.tile([L, D], BF16)
        nc.sync.dma_start(wu_b[:], wuv[0:1, :].broadcast_to([L, D]))
        aux_sb = consts.tile([L, 3 * NB], F32)
        nc.sync.dma_start(aux_sb[:], aux[:])
        rmM_all = aux_sb[:, 0:NB]          # (j<len_b ? 0 : NEG) + c
        rm1_all = aux_sb[:, NB : 2 * NB]   # [1<=j<len_b]
        rm2_all = aux_sb[:, 2 * NB : 3 * NB]  # [j<len_b-1]

        # ---- enc loads: SWDGE casting DMAs, CHUNKS batches per chunk ----
        enc_tiles = {}
        chunk_of = {}
        b0 = 0
        for ci, tb in enumerate(CHUNKS):
            chunk = enc_pool.tile([L, tb * D], BF16, tag=f"enc{ci}", name=f"encc{ci}")
            if tb == 1:
                nc.gpsimd.dma_start(chunk[:], enc[b0])
            else:
                nc.gpsimd.dma_start(
                    chunk[:].rearrange("l (b d) -> l b d", b=tb),
                    enc[b0 : b0 + tb].rearrange("b l d -> l b d"),
                )
            for i in range(tb):
                enc_tiles[b0 + i] = chunk[:, i * D : (i + 1) * D]
                chunk_of[b0 + i] = ci
            b0 += tb

        # ---- constants ----
        ones_row = consts.tile([1, L], F32)
        nc.gpsimd.memset(ones_row[:], 1.0)
        ones_col = consts.tile([L, 1], F32)
        nc.gpsimd.memset(ones_col[:], 1.0)
        ident = consts.tile([L, L], F32)
        make_identity(nc, ident[:])
        # multiplicative lower-triangular mask: tri01[j,k] = 1 if k<j else 0
        tri01 = consts.tile([L, L], F32)
        nc.gpsimd.memset(tri01[:], 1.0)
        nc.gpsimd.affine_select(
            out=tri01[:], in_=tri01[:], compare_op=ALU.is_gt, fill=0.0,
            base=0, pattern=[[-1, L]], channel_multiplier=1,
        )

        # ---- main pipeline ----
        UV = accs.tile([L, 2 * NB], F32)  # cols 0..NB-1 = u_b; NB..2NB-1 = v_b
        n_groups = len(LN_GROUPS)
        grp_of = {b: (g, q) for g, grp in enumerate(LN_GROUPS) for q, b in enumerate(grp)}
        RS = accs.tile([L, n_groups], F32)
        exg_pool = ctx.enter_context(tc.tile_pool(name="exg", bufs=1))
        exg_tiles = [exg_pool.tile([L, len(grp) * L], F32, tag=f"exg{i}", name=f"exg{i}")
                     for i, grp in enumerate(LN_GROUPS)]

        def dot_stt(enc_ap, w_tile, acc_col):
            junk = junk_pool.tile([L, D], BF16)
            nc.vector.scalar_tensor_tensor(
                out=junk[:], in0=enc_ap, scalar=1.0, op0=ALU.mult,
                in1=w_tile[:], op1=ALU.mult, accum_out=acc_col,
            )

        def dot_act(enc_ap, w_tile, acc_col):
            prod = prod_pool.tile([L, D], BF16)
            nc.vector.tensor_mul(prod[:], enc_ap, w_tile[:])
            junk = junk_pool.tile([L, D], BF16, tag="junk_act")
            nc.scalar.activation(junk[:], prod[:], ACTF.Copy, accum_out=acc_col)

        def phase_b(b):
            v_col = UV[:, NB + b : NB + b + 1]
            psum_v = psum_v_pool.tile([1, L], F32)
            nc.tensor.matmul(psum_v[:], lhsT=v_col, rhs=ident[:], is_transpose=True)
            v_row = rows_pool.tile([1, L], F32, tag="vrow")
            nc.vector.tensor_copy(v_row[:], psum_v[:])
            psum_d = psum_d_pool.tile([L, L], F32)
            nc.tensor.matmul(psum_d[:], lhsT=ones_row[:], rhs=v_row[:])
            su = rows_pool.tile([L, 1], F32, tag="su")
            nc.vector.tensor_add(su[:], UV[:, b : b + 1], rmM_all[:, b : b + 1])
            ex = d2_pool.tile([L, L], F32, tag="ex")
            nc.scalar.activation(ex[:], psum_d[:], ACTF.Exp, bias=su[:, 0:1])
            g, q = grp_of[b]
            nc.vector.tensor_mul(exg_tiles[g][:, q * L : (q + 1) * L], ex[:], tri01[:])

        def ln_group(g):
            sp = d2_pool.tile([L, len(LN_GROUPS[g]) * L], F32, tag="sp")
            nc.scalar.activation(sp[:], exg_tiles[g][:], ACTF.Ln, bias=1.0,
                                 accum_out=RS[:, g : g + 1])

        done_groups = set()
        b0 = 0
        for tb in CHUNKS:
            batches = range(b0, b0 + tb)
            for b in batches:
                # v-dots first: they gate phase B. Tail batches use DVE STT
                # so phase B isn't queued behind ACT reductions.
                if b in V_STT_SET:
                    dot_stt(enc_tiles[b], wv_b, UV[:, NB + b : NB + b + 1])
                else:
                    dot_act(enc_tiles[b], wv_b, UV[:, NB + b : NB + b + 1])
            for b in batches:
                if b in STT_SET:
                    dot_stt(enc_tiles[b], wu_b, UV[:, b : b + 1])
                else:
                    dot_act(enc_tiles[b], wu_b, UV[:, b : b + 1])
            for b in batches:
                phase_b(b)
            b0 += tb
            for g, grp in enumerate(LN_GROUPS):
                if g not in done_groups and grp[-1] < b0:
                    ln_group(g)
                    done_groups.add(g)

        # ---- diagonal (label-1) terms, all batches at once ----
        # diag sum = sum_j u[j]*rm1[j] + sum_k (v[k]+c)*rm2[k]; the c*rm2 part
        # equals c*(len-1) and is folded in on the host via rmM's c... no:
        # rm2 carries plain 0/1; vc adds nothing here because c is folded into
        # rmM (bias path). The diagonal needs v+c explicitly, so the host puts
        # c into rm2's companion: we compute sum v*rm2 and the host adds
        # c*(len_b-1) terms into its final combine.
        dUV = accs.tile([L, 2 * NB], F32)
        nc.vector.tensor_mul(dUV[:], UV[:, 0 : 2 * NB], aux_sb[:, NB : 3 * NB])

        # ---- final reduction ----
        accA = accs.tile([L, 1], F32)
        nc.vector.reduce_sum(accA[:], RS[:], axis=AXX)
        dr = accs.tile([L, 1], F32)
        nc.vector.reduce_sum(dr[:], dUV[:], axis=AXX)
        nc.vector.tensor_sub(accA[:], accA[:], dr[:])
        psum_s = psum_misc.tile([1, 1], F32, tag="psm")
        nc.tensor.matmul(psum_s[:], lhsT=accA[:], rhs=ones_col[:])
        out_t = accs.tile([1, 1], F32)
        nc.vector.tensor_copy(out_t[:], psum_s[:])
        nc.sync.dma_start(out[:], out_t[:])

    nc.compile()
    return nc


_NC = None


def _get_nc():
    global _NC
    if _NC is None:
        _NC = build_program()
    return _NC


def _prep(encoder_output, mask, W, b):
    """Host-side prep: shard + derived small tensors."""
    W = np.asarray(W, dtype=np.float32)
    b = np.asarray(b, dtype=np.float32).reshape(2)
    mask = np.asarray(mask)
    c = float(b[1] - b[0])
    wuv = np.stack([W[:D, 1] - W[:D, 0], W[D:, 1] - W[D:, 0]]).astype(ml_dtypes.bfloat16)
    lens = mask.astype(np.int64).sum(axis=1)  # [BSZ]
    j = np.arange(L)
    maps = []
    diag_c = 0.0  # host part of the diagonal c-terms: sum_b c*(len_b-1)
    for cid in range(N_CORES):
        sl = slice(cid * NB, (cid + 1) * NB)
        lc = lens[sl]  # [NB]
        rmM = np.where(j[:, None] < lc[None, :], 0.0, NEG).astype(np.float32) + c
        rm1 = ((j[:, None] >= 1) & (j[:, None] < lc[None, :])).astype(np.float32)
        rm2 = (j[:, None] < (lc[None, :] - 1)).astype(np.float32)
        aux = np.concatenate([rmM, rm1, rm2], axis=1)  # [L, 3*NB]
        maps.append(
            {
                "enc": np.ascontiguousarray(encoder_output[sl], dtype=np.float32),
                "wuv": wuv,
                "aux": np.ascontiguousarray(aux),
            }
        )
    diag_c = float(c * (lens - 1).sum())
    n_valid = int((lens * (lens - 1) // 2).sum())
    return maps, diag_c, n_valid


def kernel(encoder_output, mask, W, b, _run_kwargs=None):
    from concourse.bass_utils import run_bass_kernel_spmd

    nc = _get_nc()
    maps, diag_c, n_valid = _prep(np.asarray(encoder_output), mask, W, b)
    res = run_bass_kernel_spmd(nc, maps, core_ids=list(range(N_CORES)),
                               **(_run_kwargs or {}))
    total = float(sum(np.float64(r["out"][0, 0]) for r in res.results))
    total -= diag_c
    loss = total / max(n_valid, 1)
    out = np.array(loss, dtype=np.float32)
    if _run_kwargs is not None:
        return out, res
    return out
